# revision 14
# baseline (speedup 1.0000x reference)
"""Trainium2 Bass kernel for tanh-attention (nn_Attention_50362786513376).

reference:
  q = (x @ Wq.T) * dk^-0.5 ; k = x @ Wk.T ; v = x        (heads = 8, dk = 64)
  out = tanh(q k^T) v   per (batch, head),  merged back to [b, n, dim]

Sharding: 8 cores = 4 batches x 2 head-halves (4 heads per core).
Host pre-work (free, exact): pack x^T / v-slice / scaled-transposed weights
into SBUF-layout arrays so each device load is a handful of large DMAs.

Device per core (4 heads = 2 pairs p):
  ramp:      DMA chase -> K^T[p0] + Q^T[p0] i-block 0 projections
  steady:    per (p, iq) block, 16 j-tiles, software-pipelined:
      S^T[j,i] = K^T.T Q^T   (row-tiled pair, PE rows 0-63 / 64-127
                              run concurrently on HW)
      tanh: split between ScalarE (exact, ~2 cols/cycle on HW) and a
            custom 8-stage DVE op clip(x*(c0+s*(c1+c2*s)), +-1)
            (end-to-end rel err ~5e-3 vs tol 2e-2)
      out^T[ch,i] += v[j,ch].T tanh(S^T)  (col-tiled pair: PSUM
                              partitions 0-63 / 64-127, concurrent)
    remaining projection groups are emitted just-in-time between blocks
    so they hide inside the attention stream instead of a serial ramp.
  drain:     acc -> f16 staging -> DRAM (host casts back to f32)
Host post-work: out[b,:,half] = outT.T
"""
import numpy as np

HEADS = 8
DK = 64
B = 4
N = 2048
DIM = 512
SCALE = DK ** (-0.5)
NCORES = 8
HALF = DIM // 2  # 256 channels per core (4 heads)

_built = None
_built_cfg = None
# tanh engine split over the 128 S tiles: TANH_ACT_N tiles go to ScalarE
# (exact table tanh), the rest to the custom DVE op (approx), spread evenly
# (Bresenham). HW: ACT ~0.5ns/col, DVE custom op ~1.05ns/col.
TANH_ACT_N = 60
TRACE = False
TRACE_KW = {}
# timing aid: repeat the proj+attention phase REPS times inside the program
REPS = 1

# deg-5 odd + output clamp, fit to tanh on [0, 7.7] (max |logit| = 6.75),
# minimizing N(0,1)-weighted rms with max-err cap 0.03
TANH_C = (0.96123617, -0.21354895, 0.02356335)
_TANH_OP_NAME = "TANH5C_ANT"


def _register_tanh_op():
    """Append the custom DVE tanh op to the dve_ops registry (idempotent).

    out = clip(in0*(c0 + s*(c1 + c2*s)), -1, 1), s = in0^2 — 8 ALU stages
    (sq, mul, add, mul, add, mul, min, max; the -1 const folds), one DVE
    instruction per tile at 1 elem/cycle/lane, PSUM f32 in -> SBUF f16 out.
    """
    from concourse import dve_ops
    from concourse.dve_ops import (CUSTOM_DVE_SPECS, OPS, DveOp,
                                   _SUB_OPCODE_FOR_NAME)
    from concourse.dve_spec import (C0, C1, C2, One, Spec, Src0, Zero, lower,
                                    maxx, minn, sq)
    from concourse.dve_uop import DveOpSpec

    if _TANH_OP_NAME in _SUB_OPCODE_FOR_NAME:
        return next(o for o in OPS if o.name == _TANH_OP_NAME)

    def _ref(in0, in1, s0, s1, imm2):
        x = in0.astype(np.float32)
        s = x * x
        y = x * (s0 + s * (s1 + np.float32(imm2) * s))
        return np.clip(y, -1.0, 1.0).astype(np.float32)

    s = sq(Src0)
    y = Src0 * (C0 + s * (C1 + C2 * s))
    spec = Spec(body=maxx(minn(y, One), Zero - One), reference=_ref)
    shas = {}
    for ver in ("v3", "v4"):
        try:
            shas[ver] = DveOpSpec(name=_TANH_OP_NAME, opcode=31,
                                  uops=lower(spec, ver=ver),
                                  rd1_en=False).sha(ver)
        except Exception:
            if ver == "v3":
                raise
    op = DveOp(_TANH_OP_NAME, spec, subdim=False, uops_sha=shas)
    OPS.append(op)
    _SUB_OPCODE_FOR_NAME[_TANH_OP_NAME] = (dve_ops._CUSTOM_DVE_ROW_BASE
                                           + len(OPS) - 1)
    assert _SUB_OPCODE_FOR_NAME[_TANH_OP_NAME] < 0x20
    CUSTOM_DVE_SPECS[_TANH_OP_NAME] = spec
    return op


def _build():
    from contextlib import ExitStack

    import concourse.tile as tile
    from concourse import bacc, mybir

    tanh_op = _register_tanh_op()

    F32 = mybir.dt.float32
    F16 = mybir.dt.float16
    Tanh = mybir.ActivationFunctionType.Tanh

    nc = bacc.Bacc("TRN2", target_bir_lowering=False, debug=False,
                   num_devices=NCORES)
    # host-packed, partition-major inputs -> few large contiguous DMAs
    xT_ap = nc.dram_tensor("xT", [128, 4 * N], F16, kind="ExternalInput").ap()
    xv_ap = nc.dram_tensor("xv", [128, 16 * HALF], F16,
                           kind="ExternalInput").ap()
    wqT_ap = nc.dram_tensor("wqT", [128, 4 * HALF], F16,
                            kind="ExternalInput").ap()
    wkT_ap = nc.dram_tensor("wkT", [128, 4 * HALF], F16,
                            kind="ExternalInput").ap()
    outT_ap = nc.dram_tensor("outT", [HALF, N], F16, kind="ExternalOutput").ap()

    NJ = N // 128          # 16 j-tiles

    with tile.TileContext(nc) as tc:
        with ExitStack() as ctx:
            const = ctx.enter_context(tc.tile_pool(name="const", bufs=1))
            qk_pool = ctx.enter_context(tc.tile_pool(name="qk", bufs=1))
            tanh_pool = ctx.enter_context(tc.tile_pool(name="tanh", bufs=12))
            stg_pool = ctx.enter_context(tc.tile_pool(name="stg", bufs=4))
            warm_pool = ctx.enter_context(tc.tile_pool(name="warm", bufs=1))

            # ---- activation-table warmup: load the Tanh table during the
            # DMA ramp so the first real tanh doesn't pay ~1.3us ----
            wtile = warm_pool.tile([128, 8], F32)
            nc.gpsimd.memset(wtile[:], 0.0)
            wout = warm_pool.tile([128, 8], F16)
            nc.scalar.activation(wout[:], wtile[:], Tanh)

            # ---- input loads: weights first (projections need them first),
            # then xT in 4 ct-chunks the projections chase, then xv in 4
            # chunks the first AV j-tiles chase ----
            xT_sb = const.tile([128, 4 * N], F16)
            wq_sb = const.tile([128, 4 * HALF], F16)
            wk_sb = const.tile([128, 4 * HALF], F16)
            xv_sb = const.tile([128, 16 * HALF], F16)
            # xT is packed t4-pair-major: cols [pair*4096 + ct*1024 +
            # (t4%2)*512 + c]. Pair 0 (i/j cols 0:1024 of every ct chunk)
            # lands first as 4 chasable sub-DMAs, so the first projections
            # start after ~1MB of traffic instead of the full 2MB.
            nc.sync.dma_start(wk_sb[:], wkT_ap)
            nc.sync.dma_start(wq_sb[:], wqT_ap)
            for ct in range(4):
                nc.sync.dma_start(xT_sb[:, ct * 1024:(ct + 1) * 1024],
                                  xT_ap[:, ct * 1024:(ct + 1) * 1024])
            nc.sync.dma_start(xv_sb[:, 0:8 * HALF], xv_ap[:, 0:8 * HALF])
            nc.sync.dma_start(xT_sb[:, 4096:8192], xT_ap[:, 4096:8192])
            nc.sync.dma_start(xv_sb[:, 8 * HALF:16 * HALF],
                              xv_ap[:, 8 * HALF:16 * HALF])

            # ---- PSUM pools: 6 x [128,512] S (6 banks) + 2 x [128,512]
            # acc (2 banks); JIT projections borrow the spare acc slot ----
            QT = [qk_pool.tile([128, N], F16, tag=f"qt{p}", name=f"qt{p}")
                  for p in range(2)]
            KT = [qk_pool.tile([128, N], F16, tag=f"kt{p}", name=f"kt{p}")
                  for p in range(2)]
            ps_S = ctx.enter_context(
                tc.tile_pool(name="ps_S", bufs=3, space="PSUM"))
            ps_acc = ctx.enter_context(
                tc.tile_pool(name="ps_acc", bufs=2, space="PSUM"))

            for _rep in range(REPS):
                n_pc = 0

                def proj_mm(item, ps2, ct):
                    dst, w_sb, p, t4 = item
                    lhsT = w_sb[:, ct * HALF + p * 128:
                                ct * HALF + (p + 1) * 128]
                    rhs = xT_sb[:, (t4 // 2) * 4096 + ct * 1024
                                + (t4 % 2) * 512:
                                (t4 // 2) * 4096 + ct * 1024
                                + (t4 % 2) * 512 + 512]
                    nc.tensor.matmul(ps2[:], lhsT, rhs,
                                     start=(ct == 0), stop=(ct == 3))

                def proj_single(item):
                    """One (dst, p, t4) projection: 4 ct-chunk matmuls into
                    a spare ps_acc slot + one ACT copy to SBUF f16."""
                    dst, w_sb, p, t4 = item
                    ps2 = ps_acc.tile([128, 512], F32, tag="acc",
                                      name="proj_ps")
                    for ct in range(4):
                        proj_mm(item, ps2, ct)
                    nc.scalar.copy(dst[p][:, t4 * 512:(t4 + 1) * 512],
                                   ps2[:])

                # in-stream projections go one ct-matmul per tile so the PE
                # FIFO never pauses S production by more than ~220ns
                pend_proj = []

                def push_proj(item):
                    ps2 = ps_acc.tile([128, 512], F32, tag="acc",
                                      name="proj_ps")
                    pend_proj.extend((item, ps2, ct) for ct in range(4))

                def step_proj():
                    if not pend_proj:
                        return
                    item, ps2, ct = pend_proj.pop(0)
                    proj_mm(item, ps2, ct)
                    if ct == 3:
                        dst, _, p, t4 = item
                        nc.scalar.copy(
                            dst[p][:, t4 * 512:(t4 + 1) * 512], ps2[:])

                # ramp: only what the first QKs need — KT[p0] cols 0:1024
                # and QT[p0] i-block 0 (every proj group needs ALL xT
                # ct-chunks — ct is the contraction dim — so the ramp is
                # xT-DMA-bound; the remaining 13 groups are emitted
                # just-in-time inside the attention stream, where the PE has
                # slack under the tanh-bound steady rate).
                proj_single((KT, wk_sb, 0, 0))
                proj_single((KT, wk_sb, 0, 1))
                proj_single((QT, wq_sb, 0, 0))
                # per-block JIT singles: {block: [(at_tile, item), ...]};
                # each runs in a spare ps_acc slot ([128,512]) so the QK
                # S-tile rotation never blocks on projection PSUM
                jit_proj = {
                    0: [(2, (KT, wk_sb, 0, 2)), (8, (KT, wk_sb, 0, 3)),
                        (12, (QT, wq_sb, 0, 1))],
                    1: [(2, (QT, wq_sb, 0, 2)), (8, (KT, wk_sb, 1, 0))],
                    2: [(2, (QT, wq_sb, 0, 3)), (8, (KT, wk_sb, 1, 1))],
                    3: [(2, (KT, wk_sb, 1, 2)), (8, (KT, wk_sb, 1, 3)),
                        (12, (QT, wq_sb, 1, 0))],
                    4: [(2, (QT, wq_sb, 1, 1)), (8, (QT, wq_sb, 1, 2))],
                    5: [(2, (QT, wq_sb, 1, 3))],
                }

                # flattened stream over 128 [128,1024] score tiles (one per
                # head-pair x j-tile). KEY pipelining rule: the PE FIFO must
                # never head-block on a tanh result, so AV(g) is emitted
                # AV_LAG tiles behind QK(g+1) — by then tanh(g) finished
                # long ago and the PE free-runs, keeping the S-slot queue
                # full and both tanh engines saturated.
                AV_LAG = 4
                tiles = [(p, iq, j) for p in range(2) for iq in range(4)
                         for j in range(NJ)]
                NTL = len(tiles)
                accs = {}
                Ts = {}
                tanh_err = 0
                pend_stg = []

                def emit_stg(bi):
                    p, iq = bi // 4, bi % 4
                    acc = accs.pop(bi)
                    st = stg_pool.tile([128, 512], F16, tag="stg",
                                       name="stg")
                    nc.scalar.copy(st[:], acc[:])
                    nc.sync.dma_start(
                        outT_ap[p * 128:(p + 1) * 128,
                                iq * 512:(iq + 1) * 512],
                        st[:])

                def emit_av(g):
                    pg, piq, pj = tiles[g]
                    pbi = pg * 4 + piq
                    T_prev = Ts.pop(g)
                    if pj == 0:
                        accs[pbi] = ps_acc.tile([128, 512], F32,
                                                tag="acc", name="acc")
                    acc = accs[pbi]
                    # col-tiled pair: par0 -> PSUM partitions 0-63,
                    # par1 -> 64-127 (concurrent on HW)
                    for par in range(2):
                        v = xv_sb[:, pj * HALF + pg * 128 + par * 64:
                                  pj * HALF + pg * 128 + par * 64 + 64]
                        nc.tensor.matmul(
                            acc[par * 64:(par + 1) * 64, :],
                            v,
                            T_prev[:, par * 512:(par + 1) * 512],
                            start=(pj == 0), stop=(pj == NJ - 1),
                            tile_position=(0, par * 64))
                    if pj == NJ - 1:
                        pend_stg.append((pbi, g))

                for g in range(NTL + AV_LAG):
                    if g < NTL:
                        p, iq, j = tiles[g]
                        bi = p * 4 + iq
                        i0 = iq * 512
                        S = ps_S.tile([128, 1024], F32, tag="S", name="S")
                        # row-tiled pair: head parity 0 on PE rows 0-63,
                        # parity 1 on rows 64-127 (concurrent on HW)
                        nc.tensor.matmul(
                            S[:, 0:512],
                            KT[p][0:64, j * 128:(j + 1) * 128],
                            QT[p][0:64, i0:i0 + 512],
                            start=True, stop=True, tile_position=(0, 0))
                        nc.tensor.matmul(
                            S[:, 512:1024],
                            KT[p][64:128, j * 128:(j + 1) * 128],
                            QT[p][64:128, i0:i0 + 512],
                            start=True, stop=True, tile_position=(64, 0))
                        T = tanh_pool.tile([128, 1024], F16, tag="T",
                                           name="T")
                        tanh_err += TANH_ACT_N
                        use_act = tanh_err >= 128
                        if use_act:
                            tanh_err -= 128
                        if use_act:
                            nc.scalar.activation(T[:], S[:], Tanh)
                        else:
                            nc.vector._custom_dve(
                                tanh_op, out=T[:], in0=S[:],
                                s0=TANH_C[0], s1=TANH_C[1], imm2=TANH_C[2])
                        Ts[g] = T
                        for at_t, item in jit_proj.get(bi, ()):
                            if at_t == j:
                                push_proj(item)
                        step_proj()
                    if g >= AV_LAG:
                        emit_av(g - AV_LAG)
                    # staging copies 2 tiles after the block's last AV
                    while pend_stg and (g >= pend_stg[0][1] + AV_LAG + 2
                                        or g == NTL + AV_LAG - 1):
                        emit_stg(pend_stg.pop(0)[0])

    nc.compile()
    return nc


def _get_built():
    global _built, _built_cfg
    cfg = (TANH_ACT_N, REPS)
    if _built is None or _built_cfg != cfg:
        _built = _build()
        _built_cfg = cfg
    return _built


def kernel(x, Wq, Wk):
    from concourse.bass_utils import run_bass_kernel_spmd

    x = np.asarray(x, dtype=np.float32)
    Wq = np.asarray(Wq, dtype=np.float32)
    Wk = np.asarray(Wk, dtype=np.float32)

    nc = _get_built()
    in_maps = []
    for c in range(NCORES):
        b, half = c // 2, c % 2
        sl = slice(half * HALF, (half + 1) * HALF)
        # pair-major xT: [p, pair, ct, t4in, c]
        xT = (x[b].T.reshape(4, 128, 2, 2, 512)
              .transpose(1, 2, 0, 3, 4).reshape(128, 4 * N))
        xv = x[b][:, sl].reshape(16, 128, HALF).transpose(1, 0, 2)
        wqT = (SCALE * Wq[sl, :]).T.reshape(4, 128, HALF).transpose(1, 0, 2)
        wkT = Wk[sl, :].T.reshape(4, 128, HALF).transpose(1, 0, 2)
        in_maps.append({
            "xT": np.ascontiguousarray(xT).astype(np.float16),
            "xv": np.ascontiguousarray(xv.reshape(128, 16 * HALF)).astype(
                np.float16),
            "wqT": np.ascontiguousarray(wqT.reshape(128, 4 * HALF)).astype(
                np.float16),
            "wkT": np.ascontiguousarray(wkT.reshape(128, 4 * HALF)).astype(
                np.float16),
        })
    try:
        res = run_bass_kernel_spmd(nc, in_maps, core_ids=list(range(NCORES)),
                                   trace=TRACE, **TRACE_KW)
    except Exception:
        # transient device wedge recovers on retry
        import time as _time
        _time.sleep(2.0)
        res = run_bass_kernel_spmd(nc, in_maps, core_ids=list(range(NCORES)),
                                   trace=TRACE, **TRACE_KW)
    out = np.empty((B, N, DIM), np.float32)
    for c in range(NCORES):
        b, half = c // 2, c % 2
        out[b, :, half * HALF:(half + 1) * HALF] = (
            res.results[c]["outT"].astype(np.float32).T)
    if TRACE:
        kernel.last_results = res
    return out


# revision 15
# speedup vs baseline: 1.0112x; 1.0112x over previous
"""Trainium2 Bass kernel for tanh-attention (nn_Attention_50362786513376).

reference:
  q = (x @ Wq.T) * dk^-0.5 ; k = x @ Wk.T ; v = x        (heads = 8, dk = 64)
  out = tanh(q k^T) v   per (batch, head),  merged back to [b, n, dim]

Sharding: 8 cores = 4 batches x 2 head-halves (4 heads per core).
Host pre-work (free, exact): pack x^T / v-slice / scaled-transposed weights
into SBUF-layout arrays so each device load is a handful of large DMAs.

Device per core (4 heads = 2 pairs p):
  ramp:      DMA chase -> K^T[p0] + Q^T[p0] i-block 0 projections
  steady:    per (p, iq) block, 16 j-tiles, software-pipelined:
      S^T[j,i] = K^T.T Q^T   (row-tiled pair, PE rows 0-63 / 64-127
                              run concurrently on HW)
      tanh: split between ScalarE (exact, ~2 cols/cycle on HW) and a
            custom 8-stage DVE op clip(x*(c0+s*(c1+c2*s)), +-1)
            (end-to-end rel err ~5e-3 vs tol 2e-2)
      out^T[ch,i] += v[j,ch].T tanh(S^T)  (col-tiled pair: PSUM
                              partitions 0-63 / 64-127, concurrent)
    remaining projection groups are emitted just-in-time between blocks
    so they hide inside the attention stream instead of a serial ramp.
  drain:     acc -> f16 staging -> DRAM (host casts back to f32)
Host post-work: out[b,:,half] = outT.T
"""
import numpy as np

HEADS = 8
DK = 64
B = 4
N = 2048
DIM = 512
SCALE = DK ** (-0.5)
NCORES = 8
HALF = DIM // 2  # 256 channels per core (4 heads)

_built = None
_built_cfg = None
# tanh engine split over the 128 S tiles: TANH_ACT_N tiles go to ScalarE
# (exact table tanh), the rest to the custom DVE op (approx), spread evenly
# (Bresenham). HW: ACT ~0.5ns/col, DVE custom op ~1.05ns/col.
TANH_ACT_N = 61
TRACE = False
TRACE_KW = {}
# timing aid: repeat the proj+attention phase REPS times inside the program
REPS = 1

# deg-5 odd + output clamp, fit to tanh on [0, 7.7] (max |logit| = 6.75),
# minimizing N(0,1)-weighted rms with max-err cap 0.03
TANH_C = (0.96123617, -0.21354895, 0.02356335)
_TANH_OP_NAME = "TANH5C_ANT"


def _register_tanh_op():
    """Append the custom DVE tanh op to the dve_ops registry (idempotent).

    out = clip(in0*(c0 + s*(c1 + c2*s)), -1, 1), s = in0^2 — 8 ALU stages
    (sq, mul, add, mul, add, mul, min, max; the -1 const folds), one DVE
    instruction per tile at 1 elem/cycle/lane, PSUM f32 in -> SBUF f16 out.
    """
    from concourse import dve_ops
    from concourse.dve_ops import (CUSTOM_DVE_SPECS, OPS, DveOp,
                                   _SUB_OPCODE_FOR_NAME)
    from concourse.dve_spec import (C0, C1, C2, One, Spec, Src0, Zero, lower,
                                    maxx, minn, sq)
    from concourse.dve_uop import DveOpSpec

    if _TANH_OP_NAME in _SUB_OPCODE_FOR_NAME:
        return next(o for o in OPS if o.name == _TANH_OP_NAME)

    def _ref(in0, in1, s0, s1, imm2):
        x = in0.astype(np.float32)
        s = x * x
        y = x * (s0 + s * (s1 + np.float32(imm2) * s))
        return np.clip(y, -1.0, 1.0).astype(np.float32)

    s = sq(Src0)
    y = Src0 * (C0 + s * (C1 + C2 * s))
    spec = Spec(body=maxx(minn(y, One), Zero - One), reference=_ref)
    shas = {}
    for ver in ("v3", "v4"):
        try:
            shas[ver] = DveOpSpec(name=_TANH_OP_NAME, opcode=31,
                                  uops=lower(spec, ver=ver),
                                  rd1_en=False).sha(ver)
        except Exception:
            if ver == "v3":
                raise
    op = DveOp(_TANH_OP_NAME, spec, subdim=False, uops_sha=shas)
    OPS.append(op)
    _SUB_OPCODE_FOR_NAME[_TANH_OP_NAME] = (dve_ops._CUSTOM_DVE_ROW_BASE
                                           + len(OPS) - 1)
    assert _SUB_OPCODE_FOR_NAME[_TANH_OP_NAME] < 0x20
    CUSTOM_DVE_SPECS[_TANH_OP_NAME] = spec
    return op


def _build():
    from contextlib import ExitStack

    import concourse.tile as tile
    from concourse import bacc, mybir

    tanh_op = _register_tanh_op()

    F32 = mybir.dt.float32
    F16 = mybir.dt.float16
    Tanh = mybir.ActivationFunctionType.Tanh

    nc = bacc.Bacc("TRN2", target_bir_lowering=False, debug=False,
                   num_devices=NCORES)
    # host-packed, partition-major inputs -> few large contiguous DMAs
    xT_ap = nc.dram_tensor("xT", [128, 4 * N], F16, kind="ExternalInput").ap()
    xv_ap = nc.dram_tensor("xv", [128, 16 * HALF], F16,
                           kind="ExternalInput").ap()
    wqT_ap = nc.dram_tensor("wqT", [128, 4 * HALF], F16,
                            kind="ExternalInput").ap()
    wkT_ap = nc.dram_tensor("wkT", [128, 4 * HALF], F16,
                            kind="ExternalInput").ap()
    outT_ap = nc.dram_tensor("outT", [HALF, N], F16, kind="ExternalOutput").ap()

    NJ = N // 128          # 16 j-tiles

    with tile.TileContext(nc) as tc:
        with ExitStack() as ctx:
            const = ctx.enter_context(tc.tile_pool(name="const", bufs=1))
            qk_pool = ctx.enter_context(tc.tile_pool(name="qk", bufs=1))
            tanh_pool = ctx.enter_context(tc.tile_pool(name="tanh", bufs=10))
            stg_pool = ctx.enter_context(tc.tile_pool(name="stg", bufs=4))
            warm_pool = ctx.enter_context(tc.tile_pool(name="warm", bufs=1))

            # ---- activation-table warmup: load the Tanh table during the
            # DMA ramp so the first real tanh doesn't pay ~1.3us ----
            wtile = warm_pool.tile([128, 8], F32)
            nc.gpsimd.memset(wtile[:], 0.0)
            wout = warm_pool.tile([128, 8], F16)
            nc.scalar.activation(wout[:], wtile[:], Tanh)

            # ---- input loads: weights first (projections need them first),
            # then xT in 4 ct-chunks the projections chase, then xv in 4
            # chunks the first AV j-tiles chase ----
            xT_sb = const.tile([128, 4 * N], F16)
            wq_sb = const.tile([128, 4 * HALF], F16)
            wk_sb = const.tile([128, 4 * HALF], F16)
            xv_sb = const.tile([128, 16 * HALF], F16)
            # xT is packed t4-pair-major: cols [pair*4096 + ct*1024 +
            # (t4%2)*512 + c]. Pair 0 (i/j cols 0:1024 of every ct chunk)
            # lands first as 4 chasable sub-DMAs, so the first projections
            # start after ~1MB of traffic instead of the full 2MB.
            nc.sync.dma_start(wk_sb[:], wkT_ap)
            nc.sync.dma_start(wq_sb[:], wqT_ap)
            for ct in range(4):
                nc.sync.dma_start(xT_sb[:, ct * 1024:(ct + 1) * 1024],
                                  xT_ap[:, ct * 1024:(ct + 1) * 1024])
            nc.sync.dma_start(xv_sb[:, 0:8 * HALF], xv_ap[:, 0:8 * HALF])
            nc.sync.dma_start(xT_sb[:, 4096:8192], xT_ap[:, 4096:8192])
            nc.sync.dma_start(xv_sb[:, 8 * HALF:16 * HALF],
                              xv_ap[:, 8 * HALF:16 * HALF])

            # ---- PSUM pools: 6 x [128,512] S (6 banks) + 2 x [128,512]
            # acc (2 banks); JIT projections borrow the spare acc slot ----
            QT = [qk_pool.tile([128, N], F16, tag=f"qt{p}", name=f"qt{p}")
                  for p in range(2)]
            KT = [qk_pool.tile([128, N], F16, tag=f"kt{p}", name=f"kt{p}")
                  for p in range(2)]
            ps_S = ctx.enter_context(
                tc.tile_pool(name="ps_S", bufs=3, space="PSUM"))
            ps_acc = ctx.enter_context(
                tc.tile_pool(name="ps_acc", bufs=2, space="PSUM"))

            for _rep in range(REPS):
                n_pc = 0

                def proj_mm(item, ps2, ct):
                    dst, w_sb, p, t4 = item
                    lhsT = w_sb[:, ct * HALF + p * 128:
                                ct * HALF + (p + 1) * 128]
                    rhs = xT_sb[:, (t4 // 2) * 4096 + ct * 1024
                                + (t4 % 2) * 512:
                                (t4 // 2) * 4096 + ct * 1024
                                + (t4 % 2) * 512 + 512]
                    nc.tensor.matmul(ps2[:], lhsT, rhs,
                                     start=(ct == 0), stop=(ct == 3))

                def proj_single(item):
                    """One (dst, p, t4) projection: 4 ct-chunk matmuls into
                    a spare ps_acc slot + one ACT copy to SBUF f16."""
                    dst, w_sb, p, t4 = item
                    ps2 = ps_acc.tile([128, 512], F32, tag="acc",
                                      name="proj_ps")
                    for ct in range(4):
                        proj_mm(item, ps2, ct)
                    nc.scalar.copy(dst[p][:, t4 * 512:(t4 + 1) * 512],
                                   ps2[:])

                # in-stream projections go one ct-matmul per tile so the PE
                # FIFO never pauses S production by more than ~220ns
                pend_proj = []

                def push_proj(item):
                    ps2 = ps_acc.tile([128, 512], F32, tag="acc",
                                      name="proj_ps")
                    pend_proj.extend((item, ps2, ct) for ct in range(4))

                def step_proj():
                    if not pend_proj:
                        return
                    item, ps2, ct = pend_proj.pop(0)
                    proj_mm(item, ps2, ct)
                    if ct == 3:
                        dst, _, p, t4 = item
                        nc.scalar.copy(
                            dst[p][:, t4 * 512:(t4 + 1) * 512], ps2[:])

                # ramp: only what the first QKs need — KT[p0] cols 0:1024
                # and QT[p0] i-block 0 (every proj group needs ALL xT
                # ct-chunks — ct is the contraction dim — so the ramp is
                # xT-DMA-bound; the remaining 13 groups are emitted
                # just-in-time inside the attention stream, where the PE has
                # slack under the tanh-bound steady rate).
                proj_single((KT, wk_sb, 0, 0))
                proj_single((KT, wk_sb, 0, 1))
                proj_single((QT, wq_sb, 0, 0))
                # per-block JIT singles: {block: [(at_tile, item), ...]};
                # each runs in a spare ps_acc slot ([128,512]) so the QK
                # S-tile rotation never blocks on projection PSUM
                jit_proj = {
                    0: [(2, (KT, wk_sb, 0, 2)), (8, (KT, wk_sb, 0, 3)),
                        (12, (QT, wq_sb, 0, 1))],
                    1: [(2, (QT, wq_sb, 0, 2)), (8, (KT, wk_sb, 1, 0))],
                    2: [(2, (QT, wq_sb, 0, 3)), (8, (KT, wk_sb, 1, 1))],
                    3: [(2, (KT, wk_sb, 1, 2)), (8, (KT, wk_sb, 1, 3)),
                        (12, (QT, wq_sb, 1, 0))],
                    4: [(2, (QT, wq_sb, 1, 1)), (8, (QT, wq_sb, 1, 2))],
                    5: [(2, (QT, wq_sb, 1, 3))],
                }

                # flattened stream over 128 [128,1024] score tiles (one per
                # head-pair x j-tile). KEY pipelining rule: the PE FIFO must
                # never head-block on a tanh result, so AV(g) is emitted
                # AV_LAG tiles behind QK(g+1) — by then tanh(g) finished
                # long ago and the PE free-runs, keeping the S-slot queue
                # full and both tanh engines saturated.
                AV_LAG = 3
                tiles = [(p, iq, j) for p in range(2) for iq in range(4)
                         for j in range(NJ)]
                NTL = len(tiles)
                accs = {}
                Ts = {}
                tanh_err = 0
                pend_stg = []

                def emit_stg(bi):
                    p, iq = bi // 4, bi % 4
                    acc = accs.pop(bi)
                    st = stg_pool.tile([128, 512], F16, tag="stg",
                                       name="stg")
                    nc.scalar.copy(st[:], acc[:])
                    nc.sync.dma_start(
                        outT_ap[p * 128:(p + 1) * 128,
                                iq * 512:(iq + 1) * 512],
                        st[:])

                def emit_av(g):
                    pg, piq, pj = tiles[g]
                    pbi = pg * 4 + piq
                    T_prev = Ts.pop(g)
                    if pj == 0:
                        accs[pbi] = ps_acc.tile([128, 512], F32,
                                                tag="acc", name="acc")
                    acc = accs[pbi]
                    # col-tiled pair: par0 -> PSUM partitions 0-63,
                    # par1 -> 64-127 (concurrent on HW)
                    for par in range(2):
                        v = xv_sb[:, pj * HALF + pg * 128 + par * 64:
                                  pj * HALF + pg * 128 + par * 64 + 64]
                        nc.tensor.matmul(
                            acc[par * 64:(par + 1) * 64, :],
                            v,
                            T_prev[:, par * 512:(par + 1) * 512],
                            start=(pj == 0), stop=(pj == NJ - 1),
                            tile_position=(0, par * 64))
                    if pj == NJ - 1:
                        pend_stg.append((pbi, g))

                for g in range(NTL + AV_LAG):
                    if g < NTL:
                        p, iq, j = tiles[g]
                        bi = p * 4 + iq
                        i0 = iq * 512
                        S = ps_S.tile([128, 1024], F32, tag="S", name="S")
                        # row-tiled pair: head parity 0 on PE rows 0-63,
                        # parity 1 on rows 64-127 (concurrent on HW)
                        nc.tensor.matmul(
                            S[:, 0:512],
                            KT[p][0:64, j * 128:(j + 1) * 128],
                            QT[p][0:64, i0:i0 + 512],
                            start=True, stop=True, tile_position=(0, 0))
                        nc.tensor.matmul(
                            S[:, 512:1024],
                            KT[p][64:128, j * 128:(j + 1) * 128],
                            QT[p][64:128, i0:i0 + 512],
                            start=True, stop=True, tile_position=(64, 0))
                        T = tanh_pool.tile([128, 1024], F16, tag="T",
                                           name="T")
                        tanh_err += TANH_ACT_N
                        use_act = tanh_err >= 128
                        if use_act:
                            tanh_err -= 128
                        if use_act:
                            nc.scalar.activation(T[:], S[:], Tanh)
                        else:
                            nc.vector._custom_dve(
                                tanh_op, out=T[:], in0=S[:],
                                s0=TANH_C[0], s1=TANH_C[1], imm2=TANH_C[2])
                        Ts[g] = T
                        for at_t, item in jit_proj.get(bi, ()):
                            if at_t == j:
                                push_proj(item)
                        step_proj()
                    if g >= AV_LAG:
                        emit_av(g - AV_LAG)
                    # staging copies 2 tiles after the block's last AV
                    while pend_stg and (g >= pend_stg[0][1] + AV_LAG + 2
                                        or g == NTL + AV_LAG - 1):
                        emit_stg(pend_stg.pop(0)[0])

    nc.compile()
    return nc


def _get_built():
    global _built, _built_cfg
    cfg = (TANH_ACT_N, REPS)
    if _built is None or _built_cfg != cfg:
        _built = _build()
        _built_cfg = cfg
    return _built


def kernel(x, Wq, Wk):
    from concourse.bass_utils import run_bass_kernel_spmd

    x = np.asarray(x, dtype=np.float32)
    Wq = np.asarray(Wq, dtype=np.float32)
    Wk = np.asarray(Wk, dtype=np.float32)

    nc = _get_built()
    in_maps = []
    for c in range(NCORES):
        b, half = c // 2, c % 2
        sl = slice(half * HALF, (half + 1) * HALF)
        # pair-major xT: [p, pair, ct, t4in, c]
        xT = (x[b].T.reshape(4, 128, 2, 2, 512)
              .transpose(1, 2, 0, 3, 4).reshape(128, 4 * N))
        xv = x[b][:, sl].reshape(16, 128, HALF).transpose(1, 0, 2)
        wqT = (SCALE * Wq[sl, :]).T.reshape(4, 128, HALF).transpose(1, 0, 2)
        wkT = Wk[sl, :].T.reshape(4, 128, HALF).transpose(1, 0, 2)
        in_maps.append({
            "xT": np.ascontiguousarray(xT).astype(np.float16),
            "xv": np.ascontiguousarray(xv.reshape(128, 16 * HALF)).astype(
                np.float16),
            "wqT": np.ascontiguousarray(wqT.reshape(128, 4 * HALF)).astype(
                np.float16),
            "wkT": np.ascontiguousarray(wkT.reshape(128, 4 * HALF)).astype(
                np.float16),
        })
    try:
        res = run_bass_kernel_spmd(nc, in_maps, core_ids=list(range(NCORES)),
                                   trace=TRACE, **TRACE_KW)
    except Exception:
        # transient device wedge recovers on retry
        import time as _time
        _time.sleep(2.0)
        res = run_bass_kernel_spmd(nc, in_maps, core_ids=list(range(NCORES)),
                                   trace=TRACE, **TRACE_KW)
    out = np.empty((B, N, DIM), np.float32)
    for c in range(NCORES):
        b, half = c // 2, c % 2
        out[b, :, half * HALF:(half + 1) * HALF] = (
            res.results[c]["outT"].astype(np.float32).T)
    if TRACE:
        kernel.last_results = res
    return out
